# revision 13
# baseline (speedup 1.0000x reference)
"""Multi-head attention (B=2, N=4096, C=512, H=8, D=64) on 8 TRN2 NeuronCores.

Sharding: data-parallel over batch (2 groups of 4 cores) x tensor-parallel over
heads (2 heads/core). Per core: qkv projection, attention for its 2 heads, and
a partial output projection; the host sums the 4 per-batch partials,
transposes, adds bias.

v3 engine layout (from HW microbenchmarks):
- QK matmuls are row-tiled pairs into ONE 2-bank PSUM tile: head h0 contracts
  over SBUF partitions 0-63 into cols 0:512, h1 over partitions 64-127 into
  cols 512:1024 (tile_position (0,0)/(64,0)). The pair issues in ~233ns - the
  64-wide row tiles run concurrently - so S for both heads costs one MM.
- exp(S) alternates per key-chunk between ScalarE (exact spline exp) and
  VectorE (Schraudolph fast-exp: bf16 bits = round(S*scale*128/ln2 +
  (16256-7.5)) as one tensor_scalar mult+add writing uint16; DVE's f32->u16
  convert is round-to-nearest, HW-verified). Every softmax row mixes 50%
  exact / 50% fast chunks; measured end-to-end rel err ~7e-3.
- Softmax denominator rides the PV matmul as a ones-column appended to V
  (lhsT is [V_h | 1], M=65); the denominator row is DMA-moved to partition 0,
  inverted on VectorE, partition-broadcast on GpSimd.
- QKV projections are emitted just-in-time inside the attention stage loop and
  the PV pipeline carries across n-block boundaries: the PE never idles >1us
  mid-kernel, which keeps the HAM clock-gate at 8/8 (a >3.4us gap would
  re-throttle the PE to 1.2GHz and double every matmul for the next ~3.4us).
"""
import os
import sys

for _p in ("/opt/trn_rl_repo", "/root/.axon_site/_ro/trn_rl_repo"):
    if os.path.isdir(_p) and _p not in sys.path:
        sys.path.append(_p)

import numpy as np
from contextlib import ExitStack

import concourse.bass as bass
import concourse.mybir as mybir
import concourse.tile as tile
from concourse import bacc
from concourse.bass_utils import run_bass_kernel_spmd

F32 = mybir.dt.float32
BF16 = mybir.dt.bfloat16
U16 = mybir.dt.uint16
EXP = mybir.ActivationFunctionType.Exp
MULT = mybir.AluOpType.mult
ADD = mybir.AluOpType.add

DIM, N, HD = 512, 4096, 64
SCALE = HD ** -0.5
NB = N // 512    # 8  n-blocks of 512 queries
MB = N // 128    # 32 m-chunks of 128 keys
CC = DIM // 128  # 4  c-chunks of the model dim

LN2 = float(np.log(2.0))
C_SHIFT = 7.5
EK1 = float(SCALE * 128.0 / LN2)          # fast-exp slope (scale folded in)
EK2 = float(127.0 * 128.0 - C_SHIFT)      # fast-exp bias


def build_nc():
    nc = bacc.Bacc("TRN2", target_bir_lowering=False)
    xT = nc.declare_dram_parameter("xT", [DIM, N], F32, isOutput=False)
    wqkvT = nc.declare_dram_parameter("wqkvT", [DIM, 384], F32, isOutput=False)
    wpT = nc.declare_dram_parameter("wpT", [128, DIM], F32, isOutput=False)
    out = nc.declare_dram_parameter("out", [DIM, N], F32, isOutput=True)

    with ExitStack() as ctx:
        tc = ctx.enter_context(tile.TileContext(nc))
        big = ctx.enter_context(tc.tile_pool(name="big", bufs=1))
        stage = ctx.enter_context(tc.tile_pool(name="stage", bufs=4))
        esp = ctx.enter_context(tc.tile_pool(name="esp", bufs=6))
        ysp = ctx.enter_context(tc.tile_pool(name="ysp", bufs=3))
        nrm = ctx.enter_context(tc.tile_pool(name="nrm", bufs=4))
        ps_p = ctx.enter_context(tc.tile_pool(name="psA", bufs=2, space="PSUM"))
        po_p = ctx.enter_context(tc.tile_pool(name="psB", bufs=4, space="PSUM"))

        # ---- weight loads (staged f32 -> bf16 on GpSimd) ----
        wq = []
        for cc in range(CC):
            t = stage.tile([128, 2048], F32, tag="stage", name="stage")
            nc.sync.dma_start(out=t[:, 0:384], in_=wqkvT[cc * 128:(cc + 1) * 128, :])
            tb = big.tile([128, 384], BF16, tag=f"wqb{cc}", name=f"wqb{cc}")
            nc.gpsimd.tensor_copy(out=tb[:], in_=t[:, 0:384])
            wq.append(tb)
        t = stage.tile([128, 2048], F32, tag="stage", name="stage")
        nc.sync.dma_start(out=t[:, 0:DIM], in_=wpT[:, :])
        wpb = big.tile([128, DIM], BF16, tag="wpb", name="wpb")
        nc.gpsimd.tensor_copy(out=wpb[:], in_=t[:, 0:DIM])

        # ---- x loads: half 0 cast on DVE now; half 1 casts deferred (jit) ----
        xtb = [big.tile([128, N], BF16, tag=f"xtb{cc}", name=f"xtb{cc}")
               for cc in range(CC)]
        xstage = []
        for half in range(2):
            hs = slice(half * 2048, (half + 1) * 2048)
            for cc in range(CC):
                t = stage.tile([128, 2048], F32, tag="stage", name="stage")
                nc.sync.dma_start(out=t[:], in_=xT[cc * 128:(cc + 1) * 128, hs])
                if half == 0:
                    nc.vector.tensor_copy(out=xtb[cc][:, hs], in_=t[:])
                else:
                    xstage.append((t, cc, hs))

        # ---- persistent tiles ----
        qt = big.tile([128, N], BF16, tag="qt", name="qt")   # rows: h0 chans | h1
        kt = big.tile([128, N], BF16, tag="kt", name="kt")
        v2 = big.tile([128, 130 * MB], BF16, tag="v2", name="v2")
        nc.gpsimd.memset(v2[:], 1.0)  # ones survive; V copies skip the 65th col
        atB = big.tile([128, N], BF16, tag="atB", name="atB")
        v2v = v2[:].rearrange("p (m h w) -> p m h w", m=MB, h=2)

        # ---- jit emitters ----
        def emit_qkblk(b, which):
            # which: 0 = q (wq cols 0:128 -> qt), 1 = k (cols 128:256 -> kt)
            ns = slice(b * 512, (b + 1) * 512)
            ps = ps_p.tile([128, 1024], F32, tag="ps", name="ps")[:, 0:512]
            for cc in range(CC):
                nc.tensor.matmul(
                    ps,
                    lhsT=wq[cc][:, which * 128:(which + 1) * 128],
                    rhs=xtb[cc][:, ns],
                    start=(cc == 0),
                    stop=(cc == CC - 1),
                )
            dst = qt if which == 0 else kt
            nc.scalar.copy(out=dst[:, ns], in_=ps)

        def emit_vpair(vp):
            # chunks 2vp, 2vp+1 -> v2 (one strided DVE copy, ones untouched)
            ps = ps_p.tile([128, 1024], F32, tag="ps", name="ps")
            for j in range(2):
                mb = 2 * vp + j
                for cc in range(CC):
                    nc.tensor.matmul(
                        ps[:, j * 128:(j + 1) * 128],
                        lhsT=xtb[cc][:, mb * 128:(mb + 1) * 128],
                        rhs=wq[cc][:, 256:384],
                        start=(cc == 0),
                        stop=(cc == CC - 1),
                    )
            src = ps[:, 0:256].rearrange("p (c h w) -> p c h w", c=2, h=2)
            nc.vector.tensor_copy(
                out=v2v[:, 2 * vp:2 * vp + 2, :, 0:64], in_=src)

        def emit_proj(b):
            ns = slice(b * 512, (b + 1) * 512)
            for half in range(2):
                pp = ps_p.tile([128, 1024], F32, tag="ps", name="ps")
                for ob2 in range(2):
                    ob = half * 2 + ob2
                    nc.tensor.matmul(
                        pp[:, ob2 * 512:(ob2 + 1) * 512],
                        lhsT=wpb[:, ob * 128:(ob + 1) * 128],
                        rhs=atB[:, ns],
                        start=True,
                        stop=True,
                    )
                ys = ysp.tile([128, 1024], F32, tag="ys", name="ys")
                nc.vector.tensor_copy(out=ys[:], in_=pp[:])
                for ob2 in range(2):
                    ob = half * 2 + ob2
                    nc.sync.dma_start(
                        out=out[ob * 128:(ob + 1) * 128, ns],
                        in_=ys[:, ob2 * 512:(ob2 + 1) * 512])

        def emit_xcast(i, eng):
            t, cc, hs = xstage[i]
            if eng == "g":
                nc.gpsimd.tensor_copy(out=xtb[cc][:, hs], in_=t[:])
            else:
                nc.vector.tensor_copy(out=xtb[cc][:, hs], in_=t[:])

        # jit schedule: (nb, c) -> emissions. Stages are key-chunks now
        # (32 per nb). K-blk b needed by stage 4b; V-pair vp by stage 2vp+2;
        # Q-blk b during nb b-1; proj b-1 early in nb b.
        jit = {}
        jit[(0, 0)] = [("xg", 0), ("xv", 1)]
        jit[(0, 1)] = [("xg", 2), ("xv", 3)]
        jit[(0, 2)] = [("k", 1)]
        for vp in range(2, 16):
            jit.setdefault((0, 2 * vp - 1), []).append(("v", vp))
        jit[(0, 6)] = [("k", 2)]
        jit[(0, 10)] = [("k", 3)]
        jit[(0, 14)] = [("k", 4)]
        jit[(0, 18)] = [("k", 5)]
        jit[(0, 22)] = [("k", 6)]
        jit[(0, 26)] = [("k", 7)]
        jit[(0, 16)] = [("q", 1)]
        jit[(0, 20)] = [("q", 2)]
        jit[(0, 24)] = [("q", 3)]
        jit[(0, 28)] = [("q", 4)]
        jit[(1, 8)] = [("q", 5)]
        jit[(1, 16)] = [("q", 6)]
        jit[(1, 24)] = [("q", 7)]
        for b in range(1, NB):
            jit.setdefault((b, 12), []).append(("proj", b - 1))

        def jit_step(b, c):
            for kind, arg in jit.get((b, c), ()):
                if kind == "xg":
                    emit_xcast(arg, "g")
                elif kind == "xv":
                    emit_xcast(arg, "v")
                elif kind == "q":
                    emit_qkblk(arg, 0)
                elif kind == "k":
                    emit_qkblk(arg, 1)
                elif kind == "v":
                    emit_vpair(arg)
                elif kind == "proj":
                    emit_proj(arg)

        # ---- norm, staggered over 3 stages so no engine queue blocks on the
        # DMA->reciprocal->broadcast->mul latency chain ----
        def norm_phase1(b, po0, po1):
            # den rows to partition 0 (ScalarE copy keeps DVE clear)
            tiles = []
            for po in (po0, po1):
                yy = nrm.tile([65, 512], F32, tag="yy", name="yy")
                nc.scalar.copy(out=yy[64:65, :], in_=po[64:65, :])
                row = nrm.tile([1, 512], F32, tag="row", name="row")
                nc.sync.dma_start(out=row[:], in_=yy[64:65, :])
                tiles.append(row)
            return tiles

        def norm_phase2(rows):
            out_t = []
            for row in rows:
                rec = nrm.tile([1, 512], F32, tag="rec", name="rec")
                nc.vector.reciprocal_approx_fast(out=rec[:], in_=row[:])
                recb = nrm.tile([64, 512], F32, tag="recb", name="recb")
                nc.gpsimd.partition_broadcast(recb[:], rec[0:1, :])
                out_t.append(recb)
            return out_t

        def norm_phase3(b, po0, po1, recbs):
            ns = slice(b * 512, (b + 1) * 512)
            nc.vector.tensor_mul(out=atB[0:64, ns], in0=po0[0:64, :], in1=recbs[0][:])
            a1 = nrm.tile([64, 512], BF16, tag="a1", name="a1")
            nc.vector.tensor_mul(out=a1[:], in0=po1[0:64, :], in1=recbs[1][:])
            nc.sync.dma_start(out=atB[64:128, ns], in_=a1[:])

        # ---- prologue projections ----
        emit_qkblk(0, 0)
        emit_qkblk(0, 1)
        emit_vpair(0)
        emit_vpair(1)

        # ---- attention stage loop (stage = one 128-key chunk) ----
        def emit_exp(ps, es, use_scalar):
            if use_scalar:
                nc.scalar.activation(out=es[:], in_=ps[:], func=EXP, scale=SCALE)
            else:
                nc.vector.tensor_scalar(
                    out=es[:].bitcast(U16), in0=ps[:],
                    scalar1=EK1, scalar2=EK2, op0=MULT, op1=ADD)

        pend = []      # (b, c, es) with PV pending, depth 2
        po_of = {}     # b -> (po0, po1)
        deferred = {}  # global stage idx -> [fn]
        stage_no = [0]

        def pop_pv():
            pb, pc, es = pend.pop(0)
            if pb not in po_of:
                po_of[pb] = (
                    po_p.tile([128, 512], F32, tag="po", name="po0"),
                    po_p.tile([128, 512], F32, tag="po", name="po1"),
                )
            po0, po1 = po_of[pb]
            nc.tensor.matmul(
                po0[0:65, :], lhsT=v2v[:, pc, 0, :], rhs=es[:, 0:512],
                start=(pc == 0), stop=(pc == MB - 1))
            nc.tensor.matmul(
                po1[0:65, :], lhsT=v2v[:, pc, 1, :], rhs=es[:, 512:1024],
                start=(pc == 0), stop=(pc == MB - 1))
            if pc == MB - 1:
                del po_of[pb]
                s = stage_no[0]
                state = {}

                def p1(state=state, b=pb, po0=po0, po1=po1):
                    state["rows"] = norm_phase1(b, po0, po1)

                def p2(state=state):
                    state["recbs"] = norm_phase2(state["rows"])

                def p3(state=state, b=pb, po0=po0, po1=po1):
                    norm_phase3(b, po0, po1, state["recbs"])

                deferred.setdefault(s, []).append(p1)
                deferred.setdefault(s + 1, []).append(p2)
                deferred.setdefault(s + 2, []).append(p3)

        def run_deferred():
            for fn in deferred.pop(stage_no[0], ()):
                fn()

        for b in range(NB):
            ns = slice(b * 512, (b + 1) * 512)
            for c in range(MB):
                jit_step(b, c)
                ps = ps_p.tile([128, 1024], F32, tag="ps", name="ps")
                ks = slice(c * 128, (c + 1) * 128)
                nc.tensor.matmul(ps[:, 0:512], lhsT=kt[0:64, ks],
                                 rhs=qt[0:64, ns], start=True, stop=True)
                nc.tensor.matmul(ps[:, 512:1024], lhsT=kt[64:128, ks],
                                 rhs=qt[64:128, ns], start=True, stop=True)
                es = esp.tile([128, 1024], BF16, tag="es", name="es")
                emit_exp(ps, es, use_scalar=(c % 2 == 0 or c == 15))
                pend.append((b, c, es))
                if len(pend) > 2:
                    pop_pv()
                run_deferred()
                stage_no[0] += 1
        while pend:
            pop_pv()
            run_deferred()
            stage_no[0] += 1
        while deferred:
            run_deferred()
            stage_no[0] += 1
        emit_proj(NB - 1)

    nc.compile()
    return nc


_NC_CACHE = None
LAST_EXEC_NS = None


def kernel(x, w_qkv, w_proj, b_proj):
    global _NC_CACHE, LAST_EXEC_NS
    x = np.ascontiguousarray(np.asarray(x, dtype=np.float32))
    w_qkv = np.asarray(w_qkv, dtype=np.float32)
    w_proj = np.asarray(w_proj, dtype=np.float32)
    b_proj = np.asarray(b_proj, dtype=np.float32)
    B = x.shape[0]

    if _NC_CACHE is None:
        _NC_CACHE = build_nc()
    nc = _NC_CACHE

    xTs = [np.ascontiguousarray(x[b].T) for b in range(B)]
    in_maps = []
    for c in range(8):
        b, hp = c // 4, c % 4
        qr = w_qkv[2 * hp * 64:2 * hp * 64 + 128]
        kr = w_qkv[512 + 2 * hp * 64:512 + 2 * hp * 64 + 128]
        vr = w_qkv[1024 + 2 * hp * 64:1024 + 2 * hp * 64 + 128]
        wqkvT = np.ascontiguousarray(np.concatenate([qr, kr, vr], 0).T)
        wpT = np.ascontiguousarray(w_proj[:, hp * 128:(hp + 1) * 128].T)
        in_maps.append({"xT": xTs[b], "wqkvT": wqkvT, "wpT": wpT})

    res = run_bass_kernel_spmd(
        nc,
        in_maps,
        core_ids=list(range(8)),
        trace=bool(int(os.environ.get("ATTN_TRACE", "0"))),
    )
    LAST_EXEC_NS = res.exec_time_ns

    out = np.zeros((B, N, DIM), np.float32)
    for b in range(B):
        acc = res.results[4 * b]["out"].copy()
        for c in range(4 * b + 1, 4 * b + 4):
            acc += res.results[c]["out"]
        out[b] = acc.T + b_proj
    return out
